# revision 12
# baseline (speedup 1.0000x reference)
"""Fused multi-head attention block (QKV proj + RMSNorm + 2D RoPE + softmax
attention + out proj) for Trainium2, data-parallel over batch on 8 NeuronCores.

v3 strategy per core (one batch element, N=1024, D=1024, H=16, hd=64):
  - All PE operands bf16 (weights host-cast; x cast during the fp32 PE
    transpose drain); PSUM accumulates fp32.
  - Scores: heads processed in pairs; head A occupies SBUF partitions 0:64,
    head B 64:128 of one qkT feature tile, so the two K=64 score matmuls
    row-tile the PE array (tiles T0/T8) and run CONCURRENTLY -> 2x.
  - AV: V pair [128 keys, 64|64] col-tiles the array (psum partitions
    0:64 / 64:128 of one accumulator) -> 2x. Softmax denominators come from
    separate M=1 matmuls at pair end, 4-way col-tiled (psum rows 0/32/64/96
    of one bank, per-element has-written bits keep the four interleaved
    accumulation groups independent - HW-verified).
  - exp (128 x [128,1024] ACT ops ~= 144us) is the pacing engine; schedule
    keeps its queue fed from ~15us on: per attention pair, the next pair's
    projections/stats/rope are woven between score chunks as fill work.
  - RMSNorm stats via ones-weighted sumsq matmuls; q-side rstd applied via
    DRAM-roundtrip broadcast DMA one iteration later; k-side rstd (with
    hd^-0.5 folded) is the per-partition scale of the softmax exp ACT.
  - Out projection: bias folded in as a contraction-row matmul, PSUM
    evacuated on the (idle) scalar+vector engines, stores pipelined.
Softmax skips max-subtraction: after RMSNorm ||q||<=8, ||k||<=8 so logits
lie within [-8, 8], safely inside exp range.
"""

import sys

sys.path.insert(0, "/opt/trn_rl_repo")

import numpy as np

_BUILT = None

B, N, D = 8, 1024, 1024
H, HD = 16, 64
P = 128
NB = 2
FB = 512
KT = D // P     # 8 contraction chunks / head pairs
NT = N // P     # 8 n-chunks
THETA = 10000.0
EPS = 1e-6


def _rope_tables():
    side = int(np.sqrt(N))
    dq = HD // 4
    inv_freq = 1.0 / (THETA ** (np.arange(dq, dtype=np.float32) / dq))
    ang = np.arange(side, dtype=np.float32)[:, None] * inv_freq[None, :]
    row = np.broadcast_to(ang[:, None, :], (side, side, dq)).reshape(N, dq)
    col = np.broadcast_to(ang[None, :, :], (side, side, dq)).reshape(N, dq)
    angles = np.concatenate([row, col], axis=-1)  # [N, 32]
    return np.cos(angles), np.sin(angles)


def _build_tables():
    """cosF/sinF' [128, N] for a 2-head tile (rows: head-even dims 0..63,
    then head-odd dims 0..63). sinF'[i] carries the rotate-half sign."""
    cos, sin = _rope_tables()
    cosF = np.empty((P, N), np.float32)
    sinF = np.empty((P, N), np.float32)
    for i in range(P):
        d = i % HD
        a = d % 32
        cosF[i] = cos[:, a]
        sinF[i] = sin[:, a] * (1.0 if d < 32 else -1.0)
    return cosF, sinF


def _build_program():
    import concourse.bass as bass
    import concourse.mybir as mybir
    import concourse.tile as tile
    from concourse import bacc
    from concourse.bass import ds

    if not getattr(bacc, "_act_tables_patched", False):
        _orig_get_tables = bacc.get_activation_tables

        def _only_lnexp(arch):
            import concourse.mybir as _mb
            tabs = _orig_get_tables(arch)
            if "natural_log_exp_and_others" not in tabs:
                return tabs
            steer = set()
            for fname in ("Exp", "Ln", "Copy", "Identity", "Square"):
                steer.add(getattr(_mb.ActivationFunctionType, fname))
            out = {}
            for name, funcs in tabs.items():
                if name == "natural_log_exp_and_others":
                    out[name] = funcs
                else:
                    out[name] = funcs - steer
            return out

        bacc.get_activation_tables = _only_lnexp
        bacc._act_tables_patched = True

    BF16 = mybir.dt.bfloat16
    FP32 = mybir.dt.float32
    AF = mybir.ActivationFunctionType

    nc = bacc.Bacc("TRN2", target_bir_lowering=False, debug=False, num_devices=8)

    x = nc.dram_tensor("x", [N, D], BF16, kind="ExternalInput").ap()
    wqkt = nc.dram_tensor("wqkt", [2 * KT, P, KT, P], BF16, kind="ExternalInput").ap()
    wv_d = nc.dram_tensor("wv", [D, D], BF16, kind="ExternalInput").ap()
    ident_d = nc.dram_tensor("ident", [P, P], BF16, kind="ExternalInput").ap()
    wout = nc.dram_tensor("wout", [D, D], BF16, kind="ExternalInput").ap()
    bqkv_cols_d = nc.dram_tensor("bqkv_cols", [P, 2 * KT], FP32, kind="ExternalInput").ap()
    bv_row_d = nc.dram_tensor("bv_row", [1, D], BF16, kind="ExternalInput").ap()
    bout_row_d = nc.dram_tensor("bout_row", [1, D], BF16, kind="ExternalInput").ap()
    cosf_d = nc.dram_tensor("cosf", [P, N], BF16, kind="ExternalInput").ap()
    sinf_d = nc.dram_tensor("sinf", [P, N], BF16, kind="ExternalInput").ap()
    swap_d = nc.dram_tensor("swapm", [P, P], BF16, kind="ExternalInput").ap()
    ones2q_d = nc.dram_tensor("ones2q", [P, 2], BF16, kind="ExternalInput").ap()
    ones2k_d = nc.dram_tensor("ones2k", [P, 2], BF16, kind="ExternalInput").ap()
    out = nc.dram_tensor("out", [N, D], FP32, kind="ExternalOutput").ap()
    rstdq_d = nc.dram_tensor("rstdq_scratch", [H, N], BF16).ap()
    den_d = nc.dram_tensor("den_scratch", [H, N], FP32).ap()
    recd_d = nc.dram_tensor("rec_scratch", [H, N], BF16).ap()

    with tile.TileContext(nc) as tc:
        with tc.tile_pool(name="big", bufs=1) as big, \
             tc.tile_pool(name="tab", bufs=1) as tab, \
             tc.tile_pool(name="xin", bufs=3) as xin, \
             tc.tile_pool(name="wvo", bufs=8) as wvo, \
             tc.tile_pool(name="wqk", bufs=3) as wqkp, \
             tc.tile_pool(name="sq", bufs=2) as sqp, \
             tc.tile_pool(name="uc", bufs=4) as ucp, \
             tc.tile_pool(name="bcp", bufs=2) as bcp, \
             tc.tile_pool(name="ep", bufs=18) as ep, \
             tc.tile_pool(name="rcp", bufs=2) as rcp, \
             tc.tile_pool(name="avsp", bufs=2) as avsp, \
             tc.tile_pool(name="dbcp", bufs=4) as dbcp, \
             tc.tile_pool(name="osb", bufs=3) as osbp, \
             tc.tile_pool(name="sp", bufs=2, space="PSUM") as spp, \
             tc.tile_pool(name="av", bufs=1, space="PSUM") as avp, \
             tc.tile_pool(name="mix", bufs=2, space="PSUM") as mixp:

            xT = big.tile([P, KT, N], BF16)
            qkT = big.tile([P, 2 * KT, N], BF16)     # tile t: heads 2t,2t+1
            vnat = big.tile([P, NT, KT, P], BF16)    # [key, kchunk, pair, dim]
            oT_sb = big.tile([P, KT, N], BF16)
            rstdk = big.tile([P, 2, KT, 2], FP32)

            cosf = tab.tile([P, N], BF16)
            sinf = tab.tile([P, N], BF16)
            swapm = tab.tile([P, P], BF16)
            e0m = tab.tile([P, P], BF16)
            b128 = tab.tile([P, D], BF16)
            ones2q = tab.tile([P, 2], BF16)
            ones2k = tab.tile([P, 2], BF16)
            ones1 = tab.tile([P, 1], BF16)
            ident = tab.tile([P, P], BF16)

            bqkv_cols = tab.tile([P, 2 * KT], FP32)
            biasV = tab.tile([P, D], BF16)

            eps_t = tab.tile([P, 1], FP32)
            zero_t = tab.tile([P, 1], FP32)
            ln8_t = tab.tile([P, 1], FP32)

            nc.sync.dma_start(out=ident, in_=ident_d)
            nc.vector.memset(e0m, 0.0)
            nc.vector.memset(e0m[0:1, :], 1.0)
            nc.vector.memset(b128, 0.0)
            for dst, src in [(cosf, cosf_d), (sinf, sinf_d),
                             (swapm, swap_d),
                             (ones2q, ones2q_d), (ones2k, ones2k_d),
                             (bqkv_cols, bqkv_cols_d)]:
                nc.gpsimd.dma_start(out=dst, in_=src)
            nc.gpsimd.dma_start(out=biasV, in_=bv_row_d.broadcast_to([P, D]))
            nc.gpsimd.dma_start(out=b128[0:1, :], in_=bout_row_d)
            nc.vector.memset(ones1, 1.0)
            nc.vector.memset(eps_t, EPS)
            nc.vector.memset(zero_t, 0.0)
            nc.vector.memset(ln8_t, -2.0794415416798357)  # ln(1/8)

            # ------- phase 0: x -> xT via PE transposes (bf16) -------------
            xfs = {}

            def load_x(mc):
                xf = xin.tile([P, D], BF16, tag="xf")
                nw = 4 if mc < 2 else 2
                for q in range(nw):
                    w = D // nw
                    nc.sync.dma_start(out=xf[:, ds(q * w, w)],
                                      in_=x[ds(mc * P, P), ds(q * w, w)])
                xfs[mc] = xf

            I32 = mybir.dt.int32

            def emit_transpose(mc):
                xf = xfs.pop(mc)
                for u in range(2):
                    pxt = mixp.tile([P, 4, P], BF16, tag="mix")
                    for j in range(4):
                        nc.tensor.transpose(
                            pxt[:, j, :], xf[:, ds((u * 4 + j) * P, P)], ident)
                    nc.vector.tensor_copy(
                        out=xT[:, ds(u * 4, 4), ds(mc * P, P)].bitcast(I32),
                        in_=pxt.bitcast(I32))

            wvs = []

            def load_wv(k):
                wv = wvo.tile([P, D], BF16, tag="wvo")
                nc.gpsimd.dma_start(out=wv, in_=wv_d[ds(k * P, P), :])
                wvs.append(wv)

            def emit_vproj_half(mc, half):
                pvh = mixp.tile([P, FB], FP32, tag="mix")
                for k in range(KT):
                    nc.tensor.matmul(
                        pvh, xT[:, k, ds(mc * P, P)], wvs[k][:, ds(half * FB, FB)],
                        start=(k == 0), stop=(k == KT - 1))
                nc.vector.tensor_add(
                    out=vnat[:, mc, ds(half * 4, 4), :],
                    in0=pvh.rearrange("p (g c) -> p g c", g=4),
                    in1=biasV[:, ds(half * FB, FB)].rearrange(
                        "p (g c) -> p g c", g=4))

            # ---------------- projection / stats / rope -------------------
            def emit_proj_half(t, half):
                if half == 0:
                    wcol = wqkp.tile([P, KT, P], BF16, tag="wc")
                    nc.sync.dma_start(out=wcol, in_=wqkt[t])
                    emit_proj_half.wcol = wcol
                wcol = emit_proj_half.wcol
                pm = mixp.tile([P, FB], FP32, tag="mix")
                for k in range(KT):
                    nc.tensor.matmul(pm, wcol[:, k, :], xT[:, k, ds(half * FB, FB)],
                                     start=(k == 0), stop=(k == KT - 1))
                nc.vector.tensor_scalar_add(
                    out=qkT[:, t, ds(half * FB, FB)], in0=pm,
                    scalar1=bqkv_cols[:, t:t + 1])

            def emit_stats_q(pg):
                sq = sqp.tile([P, N], BF16, tag="sq")
                nc.vector.tensor_mul(out=sq, in0=qkT[:, pg, :], in1=qkT[:, pg, :])
                pss = mixp.tile([P, FB], FP32, tag="mix")
                nc.tensor.matmul(pss[0:2, :], ones2q, sq[:, 0:FB],
                                 start=True, stop=True)
                nc.tensor.matmul(pss[32:34, :], ones2q, sq[:, FB:N],
                                 start=True, stop=True)
                lt = rcp.tile([34, FB], FP32, tag="lt")
                nc.scalar.activation(out=lt, in_=pss[0:34, :], func=AF.Ln,
                                     scale=1.0 / HD, bias=eps_t[0:34, :])
                rq = rcp.tile([34, FB], BF16, tag="rq")
                nc.scalar.activation(out=rq, in_=lt, func=AF.Exp,
                                     scale=-0.5, bias=zero_t[0:34, :])
                nc.sync.dma_start(out=rstdq_d[2 * pg:2 * pg + 1, 0:FB],
                                  in_=rq[0:1, :])
                nc.sync.dma_start(out=rstdq_d[2 * pg + 1:2 * pg + 2, 0:FB],
                                  in_=rq[1:2, :])
                nc.sync.dma_start(out=rstdq_d[2 * pg:2 * pg + 1, FB:N],
                                  in_=rq[32:33, :])
                nc.sync.dma_start(out=rstdq_d[2 * pg + 1:2 * pg + 2, FB:N],
                                  in_=rq[33:34, :])

            def emit_stats_k(pg):
                t = KT + pg
                sq = sqp.tile([P, N], BF16, tag="sq")
                nc.vector.tensor_mul(out=sq, in0=qkT[:, t, :], in1=qkT[:, t, :])
                psT = mixp.tile([P, FB], FP32, tag="mix")
                for c in range(NT):
                    nc.tensor.matmul(psT[:, ds(c * 2, 2)], sq[:, ds(c * P, P)],
                                     ones2k, start=True, stop=True)
                lt = rcp.tile([P, 2 * NT], FP32, tag="ltk")
                nc.scalar.activation(
                    out=lt, in_=psT[:, 0:2 * NT],
                    func=AF.Ln, scale=1.0 / HD, bias=eps_t)
                nc.scalar.activation(
                    out=rstdk[:, pg % 2, :, :].rearrange("p c h -> p (c h)"),
                    in_=lt, func=AF.Exp, scale=-0.5, bias=ln8_t)

            def emit_rope(pg, kq):
                t = pg if kq == "q" else KT + pg
                qs = qkT[:, t, :]
                u = ucp.tile([P, N], BF16, tag="uc")
                c = ucp.tile([P, N], BF16, tag="uc")
                nc.vector.tensor_mul(out=u, in0=qs, in1=sinf)
                nc.vector.tensor_mul(out=c, in0=qs, in1=cosf)
                if kq == "q":
                    tmp = ucp.tile([P, N], BF16, tag="tmpq", bufs=2)
                    bcq = bcp.tile([P, N], BF16, tag="bc")
                    nc.gpsimd.dma_start(
                        out=bcq[0:HD, :],
                        in_=rstdq_d[2 * pg:2 * pg + 1, :].broadcast_to([HD, N]))
                    nc.gpsimd.dma_start(
                        out=bcq[HD:P, :],
                        in_=rstdq_d[2 * pg + 1:2 * pg + 2, :].broadcast_to([HD, N]))
                for half in range(2):
                    pr = mixp.tile([P, FB], FP32, tag="mix")
                    nc.tensor.matmul(pr, swapm, u[:, ds(half * FB, FB)],
                                     start=True, stop=True)
                    dst = qkT[:, t, ds(half * FB, FB)] if kq == "k" \
                        else tmp[:, ds(half * FB, FB)]
                    nc.vector.tensor_add(out=dst, in0=pr,
                                         in1=c[:, ds(half * FB, FB)])
                if kq == "q":
                    emit_rope.pending = (pg, tmp, bcq)

            def apply_q(pg):
                pg_, tmp, bcq = emit_rope.pending
                assert pg_ == pg
                nc.vector.tensor_mul(out=qkT[:, pg, :], in0=tmp, in1=bcq)

            # ---------------- attention pair ------------------------------
            def emit_av(pa, mc, es, last):
                if mc == 0:
                    emit_av.av = avp.tile([P, N], FP32, tag="av")
                av = emit_av.av
                st = (mc == 0)
                vA = vnat[:, mc, pa, 0:HD]
                vB = vnat[:, mc, pa, HD:P]
                eA, eB = es[(0, mc)], es[(1, mc)]
                nc.tensor.matmul(av[0:HD, 0:FB], vA, eA[:, 0:FB],
                                 start=st, stop=last)
                nc.tensor.matmul(av[HD:P, 0:FB], vB, eB[:, 0:FB],
                                 start=st, stop=last)
                nc.tensor.matmul(av[0:HD, FB:N], vA, eA[:, FB:N],
                                 start=st, stop=last)
                nc.tensor.matmul(av[HD:P, FB:N], vB, eB[:, FB:N],
                                 start=st, stop=last)

            def emit_dens(pa, es):
                d = mixp.tile([P, FB], FP32, tag="mix")
                for mc in range(NT):
                    st, lt_ = (mc == 0), (mc == NT - 1)
                    nc.tensor.matmul(d[0:1, :], ones1, es[(0, mc)][:, 0:FB],
                                     start=st, stop=lt_, tile_position=(0, 0))
                    nc.tensor.matmul(d[32:33, :], ones1, es[(0, mc)][:, FB:N],
                                     start=st, stop=lt_, tile_position=(0, 32))
                    nc.tensor.matmul(d[64:65, :], ones1, es[(1, mc)][:, 0:FB],
                                     start=st, stop=lt_, tile_position=(0, 64))
                    nc.tensor.matmul(d[96:97, :], ones1, es[(1, mc)][:, FB:N],
                                     start=st, stop=lt_, tile_position=(0, 96))
                den_sb = rcp.tile([97, FB], FP32, tag="den")
                nc.vector.tensor_copy(out=den_sb, in_=d[0:97, :])
                # rows: (A,q0),(A,q1),(B,q0),(B,q1) -> den_d[2pa:2pa+2] rows
                nc.sync.dma_start(
                    out=den_d[2 * pa:2 * pa + 2, :].rearrange(
                        "h (b q) -> (h b) q", q=FB),
                    in_=den_sb[ds(0, 4, 32), :])

            def drain_av(pa):
                avs = avsp.tile([P, N], BF16, tag="avs")
                nc.vector.tensor_copy(out=avs, in_=emit_av.av)
                drain_av.avs = avs

            def emit_recip(pa):
                dg = rcp.tile([P, 16], FP32, tag="dg")
                nc.gpsimd.dma_start(
                    out=dg,
                    in_=den_d[2 * pa:2 * pa + 2, :].rearrange(
                        "h (c q) -> (h c) q", q=16))
                rec = rcp.tile([P, 16], BF16, tag="rec")
                with nc.allow_low_precision(reason="bf16 1/den is ample"):
                    nc.vector.reciprocal(out=rec, in_=dg)
                nc.gpsimd.dma_start(
                    out=recd_d[2 * pa:2 * pa + 2, :].rearrange(
                        "h (c q) -> (h c) q", q=16),
                    in_=rec)
                avs = drain_av.avs
                for nb in range(NB):
                    dbc = dbcp.tile([P, FB], BF16, tag="dbc")
                    nc.gpsimd.dma_start(
                        out=dbc[0:HD, :],
                        in_=recd_d[2 * pa:2 * pa + 1,
                                   ds(nb * FB, FB)].broadcast_to([HD, FB]))
                    nc.gpsimd.dma_start(
                        out=dbc[HD:P, :],
                        in_=recd_d[2 * pa + 1:2 * pa + 2,
                                   ds(nb * FB, FB)].broadcast_to([HD, FB]))
                    eng = nc.gpsimd if nb == 0 else nc.vector
                    eng.tensor_mul(
                        out=oT_sb[:, pa, ds(nb * FB, FB)],
                        in0=avs[:, ds(nb * FB, FB)], in1=dbc)

            def att(pa, fill, pre_recip=None):
                apply_q(pa)
                ring = pa % 2
                es = {}
                quota = (len(fill) + NT - 1) // NT if fill else 0
                for mc in range(NT):
                    sA = spp.tile([P, N], FP32, tag="sp")
                    sB = spp.tile([P, N], FP32, tag="sp")
                    kA = qkT[0:HD, KT + pa, ds(mc * P, P)]
                    kB = qkT[HD:P, KT + pa, ds(mc * P, P)]
                    qA = qkT[0:HD, pa, :]
                    qB = qkT[HD:P, pa, :]
                    nc.tensor.matmul(sA[:, 0:FB], kA, qA[:, 0:FB],
                                     start=True, stop=True)
                    nc.tensor.matmul(sB[:, 0:FB], kB, qB[:, 0:FB],
                                     start=True, stop=True)
                    nc.tensor.matmul(sA[:, FB:N], kA, qA[:, FB:N],
                                     start=True, stop=True)
                    nc.tensor.matmul(sB[:, FB:N], kB, qB[:, FB:N],
                                     start=True, stop=True)
                    eA = ep.tile([P, N], BF16, tag="e")
                    eB = ep.tile([P, N], BF16, tag="e")
                    nc.scalar.activation(out=eA, in_=sA, func=AF.Exp,
                                         scale=rstdk[:, ring, mc, 0:1])
                    nc.scalar.activation(out=eB, in_=sB, func=AF.Exp,
                                         scale=rstdk[:, ring, mc, 1:2])
                    es[(0, mc)] = eA
                    es[(1, mc)] = eB
                    if mc > 0:
                        emit_av(pa, mc - 1, es, last=False)
                    npop = quota if quota > 1 else (2 if mc % 2 == 0 else 0)
                    for _ in range(npop):
                        if fill:
                            fill.pop(0)()
                while fill:
                    fill.pop(0)()
                emit_av(pa, NT - 1, es, last=True)
                emit_dens(pa, es)
                drain_av(pa)
                if pre_recip is not None:
                    pre_recip()
                emit_recip(pa)

            # ---------------- emission ------------------------------------
            for mc in range(3):
                load_x(mc)
            for k in range(KT):
                load_wv(k)
            for mc in range(4):
                emit_transpose(mc)
                if mc + 3 < NT:
                    load_x(mc + 3)
            emit_proj_half(0, 0)
            for mc in range(4, NT):
                emit_transpose(mc)
                if mc + 3 < NT:
                    load_x(mc + 3)
            emit_proj_half(0, 1)
            emit_stats_q(0)
            emit_rope(0, "q")
            emit_proj_half(KT + 0, 0)
            emit_proj_half(KT + 0, 1)
            emit_stats_k(0)
            emit_rope(0, "k")

            def proj_fill(pg):
                return [
                    lambda: emit_proj_half(pg, 0),
                    lambda: emit_proj_half(pg, 1),
                    lambda: emit_stats_q(pg),
                    lambda: emit_rope(pg, "q"),
                    lambda: emit_proj_half(KT + pg, 0),
                    lambda: emit_proj_half(KT + pg, 1),
                    lambda: emit_stats_k(pg),
                    lambda: emit_rope(pg, "k"),
                ]

            po_parts = {}

            def po_part(nch):
                po = spp.tile([P, N], FP32, tag="sp")
                for k in range(KT - 1):
                    och = oT_sb[:, k, ds(nch * P, P)]
                    nc.tensor.matmul(po[:, 0:FB], och, wos[k][:, 0:FB],
                                     start=(k == 0), stop=False)
                    nc.tensor.matmul(po[:, FB:N], och, wos[k][:, FB:N],
                                     start=(k == 0), stop=False)
                po_parts[nch] = po

            wos = []

            def load_wo(k):
                wo = wvo.tile([P, D], BF16, tag="wvo")
                nc.gpsimd.dma_start(out=wo, in_=wout[ds(k * P, P), :])
                wos.append(wo)

            for pa in range(KT):
                fill = []
                if pa == 0:
                    for mc in range(NT):
                        fill.append(lambda mc=mc: emit_vproj_half(mc, 0))
                        fill.append(lambda mc=mc: emit_vproj_half(mc, 1))
                if pa + 1 < KT:
                    fill.extend(proj_fill(pa + 1))
                if pa == KT - 2:
                    fill.append(lambda: [load_wo(k) for k in range(KT)])
                if pa == KT - 1:
                    att(pa, fill,
                        pre_recip=lambda: (po_part(0), po_part(1)))
                else:
                    att(pa, fill)

            # ---------------- out projection ------------------------------
            for nch in range(NT):
                po = po_parts.pop(nch, None)
                if po is None:
                    po = spp.tile([P, N], FP32, tag="sp")
                    ks = range(KT)
                else:
                    ks = range(KT - 1, KT)
                for k in ks:
                    och = oT_sb[:, k, ds(nch * P, P)]
                    nc.tensor.matmul(po[:, 0:FB], och, wos[k][:, 0:FB],
                                     start=(k == 0), stop=False)
                    nc.tensor.matmul(po[:, FB:N], och, wos[k][:, FB:N],
                                     start=(k == 0), stop=False)
                nc.tensor.matmul(po[:, 0:FB], e0m, b128[:, 0:FB],
                                 start=False, stop=True)
                nc.tensor.matmul(po[:, FB:N], e0m, b128[:, FB:N],
                                 start=False, stop=True)
                for half in range(2):
                    osbt = osbp.tile([P, FB], FP32, tag="osb")
                    if half == 0:
                        nc.scalar.copy(out=osbt, in_=po[:, 0:FB])
                    else:
                        nc.vector.tensor_copy(out=osbt, in_=po[:, FB:N])
                    eng = (nc.sync, nc.gpsimd, nc.scalar)[(2 * nch + half) % 3]
                    eng.dma_start(out=out[ds(nch * P, P), ds(half * FB, FB)],
                                  in_=osbt)

    nc.compile()
    return nc


def _host_inputs(Wqkv, bqkv, Wout, bout, q_scale, k_scale):
    import ml_dtypes
    BF = ml_dtypes.bfloat16
    cosF, sinF = _build_tables()

    swapm = np.zeros((P, P), np.float32)
    for k in range(P):
        m = (k & ~63) + ((k & 63) ^ 32)
        swapm[k, m] = 1.0

    # Fold q/k_scale into the Q/K projection columns; the RMSNorm variance of
    # the *unscaled* q is then recovered with a 1/scale^2-weighted reduction.
    qs = q_scale.astype(np.float32)
    ks = k_scale.astype(np.float32)
    W = Wqkv.astype(np.float32).copy()
    b = bqkv.astype(np.float32).copy()
    qcol = np.tile(qs, H)
    kcol = np.tile(ks, H)
    W[:, 0:D] *= qcol[None, :]
    W[:, D:2 * D] *= kcol[None, :]
    b[0:D] *= qcol
    b[D:2 * D] *= kcol

    def wones(sv):
        o = np.zeros((P, 2), np.float32)
        inv2 = 1.0 / (sv * sv)
        o[0:HD, 0] = inv2
        o[HD:P, 1] = inv2
        return o

    bqkv_cols = np.ascontiguousarray(
        b[:2 * D].reshape(2 * KT, P).T).astype(np.float32)

    Wqk = W[:, :2 * D].astype(BF)
    # wqkt[t, ki, ko, f] = Wqk[ko*128+ki, t*128+f]
    wqkt = np.ascontiguousarray(
        Wqk.reshape(KT, P, 2 * KT, P).transpose(2, 1, 0, 3))

    return {
        "wqkt": wqkt,
        "wv": np.ascontiguousarray(W[:, 2 * D:]).astype(BF),
        "wout": Wout.astype(np.float32).astype(BF),
        "bqkv_cols": bqkv_cols,
        "bv_row": b[2 * D:].reshape(1, D).astype(BF),
        "bout_row": bout.reshape(1, D).astype(np.float32).astype(BF),
        "cosf": cosF.astype(BF), "sinf": sinF.astype(BF),
        "swapm": swapm.astype(BF),
        "ones2q": wones(qs).astype(BF), "ones2k": wones(ks).astype(BF),
        "ident": np.eye(P, dtype=np.float32).astype(BF),
    }


def _get_built():
    global _BUILT
    if _BUILT is None:
        _BUILT = _build_program()
    return _BUILT


def kernel(x, Wqkv, bqkv, Wout, bout, q_scale, k_scale, _trace=False):
    from concourse.bass_utils import run_bass_kernel_spmd

    import ml_dtypes
    x = np.asarray(x, dtype=np.float32).astype(ml_dtypes.bfloat16)
    shared = _host_inputs(np.asarray(Wqkv, np.float32), np.asarray(bqkv, np.float32),
                          np.asarray(Wout, np.float32), np.asarray(bout, np.float32),
                          np.asarray(q_scale, np.float32), np.asarray(k_scale, np.float32))
    in_maps = [dict(shared, x=np.ascontiguousarray(x[c])) for c in range(B)]
    nc = _get_built()
    res = run_bass_kernel_spmd(nc, in_maps, core_ids=list(range(B)), trace=_trace)
    out = np.stack([res.results[c]["out"] for c in range(B)], axis=0)
    kernel.last_exec_time_ns = res.exec_time_ns
    kernel.last_results = res
    return out


# revision 13
# speedup vs baseline: 1.0733x; 1.0733x over previous
"""Fused multi-head attention block (QKV proj + RMSNorm + 2D RoPE + softmax
attention + out proj) for Trainium2, data-parallel over batch on 8 NeuronCores.

v3 strategy per core (one batch element, N=1024, D=1024, H=16, hd=64):
  - All PE operands bf16 (weights host-cast; x cast during the fp32 PE
    transpose drain); PSUM accumulates fp32.
  - Scores: heads processed in pairs; head A occupies SBUF partitions 0:64,
    head B 64:128 of one qkT feature tile, so the two K=64 score matmuls
    row-tile the PE array (tiles T0/T8) and run CONCURRENTLY -> 2x.
  - AV: V pair [128 keys, 64|64] col-tiles the array (psum partitions
    0:64 / 64:128 of one accumulator) -> 2x. Softmax denominators come from
    separate M=1 matmuls at pair end, 4-way col-tiled (psum rows 0/32/64/96
    of one bank, per-element has-written bits keep the four interleaved
    accumulation groups independent - HW-verified).
  - exp (128 x [128,1024] ACT ops ~= 144us) is the pacing engine; schedule
    keeps its queue fed from ~15us on: per attention pair, the next pair's
    projections/stats/rope are woven between score chunks as fill work.
  - RMSNorm stats via ones-weighted sumsq matmuls; q-side rstd applied via
    DRAM-roundtrip broadcast DMA one iteration later; k-side rstd (with
    hd^-0.5 folded) is the per-partition scale of the softmax exp ACT.
  - Out projection: bias folded in as a contraction-row matmul, PSUM
    evacuated on the (idle) scalar+vector engines, stores pipelined.
Softmax skips max-subtraction: after RMSNorm ||q||<=8, ||k||<=8 so logits
lie within [-8, 8], safely inside exp range.
"""

import sys

sys.path.insert(0, "/opt/trn_rl_repo")

import numpy as np

_BUILT = None

B, N, D = 8, 1024, 1024
H, HD = 16, 64
P = 128
NB = 2
FB = 512
KT = D // P     # 8 contraction chunks / head pairs
NT = N // P     # 8 n-chunks
THETA = 10000.0
EPS = 1e-6


def _rope_tables():
    side = int(np.sqrt(N))
    dq = HD // 4
    inv_freq = 1.0 / (THETA ** (np.arange(dq, dtype=np.float32) / dq))
    ang = np.arange(side, dtype=np.float32)[:, None] * inv_freq[None, :]
    row = np.broadcast_to(ang[:, None, :], (side, side, dq)).reshape(N, dq)
    col = np.broadcast_to(ang[None, :, :], (side, side, dq)).reshape(N, dq)
    angles = np.concatenate([row, col], axis=-1)  # [N, 32]
    return np.cos(angles), np.sin(angles)


def _build_tables():
    """cosF/sinF' [128, N] for a 2-head tile (rows: head-even dims 0..63,
    then head-odd dims 0..63). sinF'[i] carries the rotate-half sign."""
    cos, sin = _rope_tables()
    cosF = np.empty((P, N), np.float32)
    sinF = np.empty((P, N), np.float32)
    for i in range(P):
        d = i % HD
        a = d % 32
        cosF[i] = cos[:, a]
        sinF[i] = sin[:, a] * (1.0 if d < 32 else -1.0)
    return cosF, sinF


def _build_program():
    import concourse.bass as bass
    import concourse.mybir as mybir
    import concourse.tile as tile
    from concourse import bacc
    from concourse.bass import ds

    if not getattr(bacc, "_act_tables_patched", False):
        _orig_get_tables = bacc.get_activation_tables

        def _only_lnexp(arch):
            import concourse.mybir as _mb
            tabs = _orig_get_tables(arch)
            if "natural_log_exp_and_others" not in tabs:
                return tabs
            steer = set()
            for fname in ("Exp", "Ln", "Copy", "Identity", "Square"):
                steer.add(getattr(_mb.ActivationFunctionType, fname))
            out = {}
            for name, funcs in tabs.items():
                if name == "natural_log_exp_and_others":
                    out[name] = funcs
                else:
                    out[name] = funcs - steer
            return out

        bacc.get_activation_tables = _only_lnexp
        bacc._act_tables_patched = True

    BF16 = mybir.dt.bfloat16
    FP32 = mybir.dt.float32
    AF = mybir.ActivationFunctionType

    nc = bacc.Bacc("TRN2", target_bir_lowering=False, debug=False, num_devices=8)

    x = nc.dram_tensor("x", [N, D], BF16, kind="ExternalInput").ap()
    wqkt = nc.dram_tensor("wqkt", [2 * KT, P, KT, P], BF16, kind="ExternalInput").ap()
    wv_d = nc.dram_tensor("wv", [D, D], BF16, kind="ExternalInput").ap()
    ident_d = nc.dram_tensor("ident", [P, P], BF16, kind="ExternalInput").ap()
    wout = nc.dram_tensor("wout", [D, D], BF16, kind="ExternalInput").ap()
    bqkv_cols_d = nc.dram_tensor("bqkv_cols", [P, 2 * KT], FP32, kind="ExternalInput").ap()
    bv_row_d = nc.dram_tensor("bv_row", [1, D], BF16, kind="ExternalInput").ap()
    bout_row_d = nc.dram_tensor("bout_row", [1, D], BF16, kind="ExternalInput").ap()
    cosf_d = nc.dram_tensor("cosf", [P, N], BF16, kind="ExternalInput").ap()
    sinf_d = nc.dram_tensor("sinf", [P, N], BF16, kind="ExternalInput").ap()
    swap_d = nc.dram_tensor("swapm", [P, P], BF16, kind="ExternalInput").ap()
    ones2q_d = nc.dram_tensor("ones2q", [P, 2], BF16, kind="ExternalInput").ap()
    ones2k_d = nc.dram_tensor("ones2k", [P, 2], BF16, kind="ExternalInput").ap()
    out = nc.dram_tensor("out", [N, D], FP32, kind="ExternalOutput").ap()
    rstdq_d = nc.dram_tensor("rstdq_scratch", [H, N], BF16).ap()
    den_d = nc.dram_tensor("den_scratch", [H, N], FP32).ap()
    recd_d = nc.dram_tensor("rec_scratch", [H, N], BF16).ap()

    with tile.TileContext(nc) as tc:
        with tc.tile_pool(name="big", bufs=1) as big, \
             tc.tile_pool(name="tab", bufs=1) as tab, \
             tc.tile_pool(name="xin", bufs=3) as xin, \
             tc.tile_pool(name="wvo", bufs=8) as wvo, \
             tc.tile_pool(name="wqk", bufs=3) as wqkp, \
             tc.tile_pool(name="sq", bufs=2) as sqp, \
             tc.tile_pool(name="uc", bufs=4) as ucp, \
             tc.tile_pool(name="bcp", bufs=2) as bcp, \
             tc.tile_pool(name="ep", bufs=18) as ep, \
             tc.tile_pool(name="rcp", bufs=2) as rcp, \
             tc.tile_pool(name="avsp", bufs=2) as avsp, \
             tc.tile_pool(name="dbcp", bufs=4) as dbcp, \
             tc.tile_pool(name="osb", bufs=3) as osbp, \
             tc.tile_pool(name="sp", bufs=2, space="PSUM") as spp, \
             tc.tile_pool(name="av", bufs=1, space="PSUM") as avp, \
             tc.tile_pool(name="mix", bufs=2, space="PSUM") as mixp:

            xT = big.tile([P, KT, N], BF16)
            qkT = big.tile([P, 2 * KT, N], BF16)     # tile t: heads 2t,2t+1
            vnat = big.tile([P, NT, KT, P], BF16)    # [key, kchunk, pair, dim]
            oT_sb = big.tile([P, KT, N], BF16)
            rstdk = big.tile([P, 2, KT, 2], FP32)

            cosf = tab.tile([P, N], BF16)
            sinf = tab.tile([P, N], BF16)
            swapm = tab.tile([P, P], BF16)
            e0m = tab.tile([P, P], BF16)
            b128 = tab.tile([P, D], BF16)
            ones2q = tab.tile([P, 2], BF16)
            ones2k = tab.tile([P, 2], BF16)
            ones1 = tab.tile([P, 1], BF16)
            ident = tab.tile([P, P], BF16)

            bqkv_cols = tab.tile([P, 2 * KT], FP32)
            biasV = tab.tile([P, D], BF16)

            eps_t = tab.tile([P, 1], FP32)
            zero_t = tab.tile([P, 1], FP32)
            ln8_t = tab.tile([P, 1], FP32)

            nc.sync.dma_start(out=ident, in_=ident_d)
            nc.vector.memset(e0m, 0.0)
            nc.vector.memset(e0m[0:1, :], 1.0)
            nc.vector.memset(b128, 0.0)
            for dst, src in [(cosf, cosf_d), (sinf, sinf_d),
                             (swapm, swap_d),
                             (ones2q, ones2q_d), (ones2k, ones2k_d),
                             (bqkv_cols, bqkv_cols_d)]:
                nc.gpsimd.dma_start(out=dst, in_=src)
            nc.gpsimd.dma_start(out=biasV, in_=bv_row_d.broadcast_to([P, D]))
            nc.gpsimd.dma_start(out=b128[0:1, :], in_=bout_row_d)
            nc.vector.memset(ones1, 1.0)
            nc.vector.memset(eps_t, EPS)
            nc.vector.memset(zero_t, 0.0)
            nc.vector.memset(ln8_t, -2.0794415416798357)  # ln(1/8)

            # ------- phase 0: x -> xT via PE transposes (bf16) -------------
            xfs = {}

            def load_x(mc):
                xf = xin.tile([P, D], BF16, tag="xf")
                nw = 4 if mc < 2 else 2
                for q in range(nw):
                    w = D // nw
                    eng = nc.sync if q % 2 == 0 else nc.scalar
                    eng.dma_start(out=xf[:, ds(q * w, w)],
                                  in_=x[ds(mc * P, P), ds(q * w, w)])
                xfs[mc] = xf

            I32 = mybir.dt.int32

            def emit_transpose(mc):
                xf = xfs.pop(mc)
                for u in range(2):
                    pxt = mixp.tile([P, 4, P], BF16, tag="mix")
                    for j in range(4):
                        nc.tensor.transpose(
                            pxt[:, j, :], xf[:, ds((u * 4 + j) * P, P)], ident)
                    nc.vector.tensor_copy(
                        out=xT[:, ds(u * 4, 4), ds(mc * P, P)].bitcast(I32),
                        in_=pxt.bitcast(I32))

            wvs = []

            def load_wv(k):
                wv = wvo.tile([P, D], BF16, tag="wvo")
                nc.gpsimd.dma_start(out=wv, in_=wv_d[ds(k * P, P), :])
                wvs.append(wv)

            def emit_vproj_half(mc, half):
                pvh = mixp.tile([P, FB], FP32, tag="mix")
                for k in range(KT):
                    nc.tensor.matmul(
                        pvh, xT[:, k, ds(mc * P, P)], wvs[k][:, ds(half * FB, FB)],
                        start=(k == 0), stop=(k == KT - 1))
                nc.vector.tensor_add(
                    out=vnat[:, mc, ds(half * 4, 4), :],
                    in0=pvh.rearrange("p (g c) -> p g c", g=4),
                    in1=biasV[:, ds(half * FB, FB)].rearrange(
                        "p (g c) -> p g c", g=4))

            # ---------------- projection / stats / rope -------------------
            def emit_proj_half(t, half):
                if half == 0:
                    wcol = wqkp.tile([P, KT, P], BF16, tag="wc")
                    nc.sync.dma_start(out=wcol, in_=wqkt[t])
                    emit_proj_half.wcol = wcol
                wcol = emit_proj_half.wcol
                pm = mixp.tile([P, FB], FP32, tag="mix")
                for k in range(KT):
                    nc.tensor.matmul(pm, wcol[:, k, :], xT[:, k, ds(half * FB, FB)],
                                     start=(k == 0), stop=(k == KT - 1))
                nc.vector.tensor_scalar_add(
                    out=qkT[:, t, ds(half * FB, FB)], in0=pm,
                    scalar1=bqkv_cols[:, t:t + 1])

            def emit_stats_q(pg):
                sq = sqp.tile([P, N], BF16, tag="sq")
                nc.vector.tensor_mul(out=sq, in0=qkT[:, pg, :], in1=qkT[:, pg, :])
                pss = mixp.tile([P, FB], FP32, tag="mix")
                nc.tensor.matmul(pss[0:2, :], ones2q, sq[:, 0:FB],
                                 start=True, stop=True)
                nc.tensor.matmul(pss[32:34, :], ones2q, sq[:, FB:N],
                                 start=True, stop=True)
                lt = rcp.tile([34, FB], FP32, tag="lt")
                nc.scalar.activation(out=lt, in_=pss[0:34, :], func=AF.Ln,
                                     scale=1.0 / HD, bias=eps_t[0:34, :])
                rq = rcp.tile([34, FB], BF16, tag="rq")
                nc.scalar.activation(out=rq, in_=lt, func=AF.Exp,
                                     scale=-0.5, bias=zero_t[0:34, :])
                nc.sync.dma_start(out=rstdq_d[2 * pg:2 * pg + 1, 0:FB],
                                  in_=rq[0:1, :])
                nc.sync.dma_start(out=rstdq_d[2 * pg + 1:2 * pg + 2, 0:FB],
                                  in_=rq[1:2, :])
                nc.sync.dma_start(out=rstdq_d[2 * pg:2 * pg + 1, FB:N],
                                  in_=rq[32:33, :])
                nc.sync.dma_start(out=rstdq_d[2 * pg + 1:2 * pg + 2, FB:N],
                                  in_=rq[33:34, :])

            def emit_stats_k(pg):
                t = KT + pg
                sq = sqp.tile([P, N], BF16, tag="sq")
                nc.vector.tensor_mul(out=sq, in0=qkT[:, t, :], in1=qkT[:, t, :])
                psT = mixp.tile([P, FB], FP32, tag="mix")
                for c in range(NT):
                    nc.tensor.matmul(psT[:, ds(c * 2, 2)], sq[:, ds(c * P, P)],
                                     ones2k, start=True, stop=True)
                lt = rcp.tile([P, 2 * NT], FP32, tag="ltk")
                nc.scalar.activation(
                    out=lt, in_=psT[:, 0:2 * NT],
                    func=AF.Ln, scale=1.0 / HD, bias=eps_t)
                nc.scalar.activation(
                    out=rstdk[:, pg % 2, :, :].rearrange("p c h -> p (c h)"),
                    in_=lt, func=AF.Exp, scale=-0.5, bias=ln8_t)

            def emit_rope(pg, kq):
                t = pg if kq == "q" else KT + pg
                qs = qkT[:, t, :]
                u = ucp.tile([P, N], BF16, tag="uc")
                c = ucp.tile([P, N], BF16, tag="uc")
                nc.vector.tensor_mul(out=u, in0=qs, in1=sinf)
                nc.vector.tensor_mul(out=c, in0=qs, in1=cosf)
                if kq == "q":
                    tmp = ucp.tile([P, N], BF16, tag="tmpq", bufs=2)
                    bcq = bcp.tile([P, N], BF16, tag="bc")
                    nc.sync.dma_start(
                        out=bcq[0:HD, :],
                        in_=rstdq_d[2 * pg:2 * pg + 1, :].broadcast_to([HD, N]))
                    nc.sync.dma_start(
                        out=bcq[HD:P, :],
                        in_=rstdq_d[2 * pg + 1:2 * pg + 2, :].broadcast_to([HD, N]))
                for half in range(2):
                    pr = mixp.tile([P, FB], FP32, tag="mix")
                    nc.tensor.matmul(pr, swapm, u[:, ds(half * FB, FB)],
                                     start=True, stop=True)
                    dst = qkT[:, t, ds(half * FB, FB)] if kq == "k" \
                        else tmp[:, ds(half * FB, FB)]
                    nc.vector.tensor_add(out=dst, in0=pr,
                                         in1=c[:, ds(half * FB, FB)])
                if kq == "q":
                    emit_rope.pending = (pg, tmp, bcq)

            def apply_q(pg):
                pg_, tmp, bcq = emit_rope.pending
                assert pg_ == pg
                nc.vector.tensor_mul(out=qkT[:, pg, :], in0=tmp, in1=bcq)

            # ---------------- attention pair ------------------------------
            def emit_av(pa, mc, es, last):
                if mc == 0:
                    emit_av.av = avp.tile([P, N], FP32, tag="av")
                av = emit_av.av
                st = (mc == 0)
                vA = vnat[:, mc, pa, 0:HD]
                vB = vnat[:, mc, pa, HD:P]
                eA, eB = es[(0, mc)], es[(1, mc)]
                nc.tensor.matmul(av[0:HD, 0:FB], vA, eA[:, 0:FB],
                                 start=st, stop=last)
                nc.tensor.matmul(av[HD:P, 0:FB], vB, eB[:, 0:FB],
                                 start=st, stop=last)
                nc.tensor.matmul(av[0:HD, FB:N], vA, eA[:, FB:N],
                                 start=st, stop=last)
                nc.tensor.matmul(av[HD:P, FB:N], vB, eB[:, FB:N],
                                 start=st, stop=last)

            def emit_dens(pa, es):
                d = mixp.tile([P, FB], FP32, tag="mix")
                for mc in range(NT):
                    st, lt_ = (mc == 0), (mc == NT - 1)
                    nc.tensor.matmul(d[0:1, :], ones1, es[(0, mc)][:, 0:FB],
                                     start=st, stop=lt_, tile_position=(0, 0))
                    nc.tensor.matmul(d[32:33, :], ones1, es[(0, mc)][:, FB:N],
                                     start=st, stop=lt_, tile_position=(0, 32))
                    nc.tensor.matmul(d[64:65, :], ones1, es[(1, mc)][:, 0:FB],
                                     start=st, stop=lt_, tile_position=(0, 64))
                    nc.tensor.matmul(d[96:97, :], ones1, es[(1, mc)][:, FB:N],
                                     start=st, stop=lt_, tile_position=(0, 96))
                den_sb = rcp.tile([97, FB], FP32, tag="den")
                nc.vector.tensor_copy(out=den_sb, in_=d[0:97, :])
                # rows: (A,q0),(A,q1),(B,q0),(B,q1) -> den_d[2pa:2pa+2] rows
                nc.sync.dma_start(
                    out=den_d[2 * pa:2 * pa + 2, :].rearrange(
                        "h (b q) -> (h b) q", q=FB),
                    in_=den_sb[ds(0, 4, 32), :])

            def drain_av(pa):
                avs = avsp.tile([P, N], BF16, tag="avs")
                nc.vector.tensor_copy(out=avs, in_=emit_av.av)
                drain_av.avs = avs

            def emit_recip(pa):
                dg = rcp.tile([P, 16], FP32, tag="dg")
                nc.sync.dma_start(
                    out=dg,
                    in_=den_d[2 * pa:2 * pa + 2, :].rearrange(
                        "h (c q) -> (h c) q", q=16))
                rec = rcp.tile([P, 16], BF16, tag="rec")
                with nc.allow_low_precision(reason="bf16 1/den is ample"):
                    nc.vector.reciprocal(out=rec, in_=dg)
                nc.sync.dma_start(
                    out=recd_d[2 * pa:2 * pa + 2, :].rearrange(
                        "h (c q) -> (h c) q", q=16),
                    in_=rec)
                avs = drain_av.avs
                for nb in range(NB):
                    dbc = dbcp.tile([P, FB], BF16, tag="dbc")
                    nc.sync.dma_start(
                        out=dbc[0:HD, :],
                        in_=recd_d[2 * pa:2 * pa + 1,
                                   ds(nb * FB, FB)].broadcast_to([HD, FB]))
                    nc.sync.dma_start(
                        out=dbc[HD:P, :],
                        in_=recd_d[2 * pa + 1:2 * pa + 2,
                                   ds(nb * FB, FB)].broadcast_to([HD, FB]))
                    eng = nc.gpsimd if nb == 0 else nc.vector
                    eng.tensor_mul(
                        out=oT_sb[:, pa, ds(nb * FB, FB)],
                        in0=avs[:, ds(nb * FB, FB)], in1=dbc)

            def att(pa, fill, pre_recip=None):
                apply_q(pa)
                ring = pa % 2
                es = {}
                quota = (len(fill) + NT - 1) // NT if fill else 0
                for mc in range(NT):
                    sA = spp.tile([P, N], FP32, tag="sp")
                    sB = spp.tile([P, N], FP32, tag="sp")
                    kA = qkT[0:HD, KT + pa, ds(mc * P, P)]
                    kB = qkT[HD:P, KT + pa, ds(mc * P, P)]
                    qA = qkT[0:HD, pa, :]
                    qB = qkT[HD:P, pa, :]
                    nc.tensor.matmul(sA[:, 0:FB], kA, qA[:, 0:FB],
                                     start=True, stop=True)
                    nc.tensor.matmul(sB[:, 0:FB], kB, qB[:, 0:FB],
                                     start=True, stop=True)
                    nc.tensor.matmul(sA[:, FB:N], kA, qA[:, FB:N],
                                     start=True, stop=True)
                    nc.tensor.matmul(sB[:, FB:N], kB, qB[:, FB:N],
                                     start=True, stop=True)
                    eA = ep.tile([P, N], BF16, tag="e")
                    eB = ep.tile([P, N], BF16, tag="e")
                    nc.scalar.activation(out=eA, in_=sA, func=AF.Exp,
                                         scale=rstdk[:, ring, mc, 0:1])
                    nc.scalar.activation(out=eB, in_=sB, func=AF.Exp,
                                         scale=rstdk[:, ring, mc, 1:2])
                    es[(0, mc)] = eA
                    es[(1, mc)] = eB
                    if mc > 0:
                        emit_av(pa, mc - 1, es, last=False)
                    for _ in range(max(quota, 1)):
                        if fill:
                            fill.pop(0)()
                while fill:
                    fill.pop(0)()
                emit_av(pa, NT - 1, es, last=True)
                emit_dens(pa, es)
                drain_av(pa)
                if pre_recip is not None:
                    pre_recip()
                emit_recip(pa)

            # ---------------- emission ------------------------------------
            for mc in range(3):
                load_x(mc)
            for k in range(KT):
                load_wv(k)
            for mc in range(4):
                emit_transpose(mc)
                if mc + 3 < NT:
                    load_x(mc + 3)
            emit_proj_half(0, 0)
            for mc in range(4, NT):
                emit_transpose(mc)
                if mc + 3 < NT:
                    load_x(mc + 3)
            emit_proj_half(0, 1)
            emit_stats_q(0)
            emit_rope(0, "q")
            emit_proj_half(KT + 0, 0)
            emit_proj_half(KT + 0, 1)
            emit_stats_k(0)
            emit_rope(0, "k")

            def proj_fill(pg):
                return [
                    lambda: emit_proj_half(pg, 0),
                    lambda: emit_proj_half(pg, 1),
                    lambda: emit_stats_q(pg),
                    lambda: emit_rope(pg, "q"),
                    lambda: emit_proj_half(KT + pg, 0),
                    lambda: emit_proj_half(KT + pg, 1),
                    lambda: emit_stats_k(pg),
                    lambda: emit_rope(pg, "k"),
                ]

            po_parts = {}

            def po_part(nch):
                po = spp.tile([P, N], FP32, tag="sp")
                for k in range(KT - 1):
                    och = oT_sb[:, k, ds(nch * P, P)]
                    nc.tensor.matmul(po[:, 0:FB], och, wos[k][:, 0:FB],
                                     start=(k == 0), stop=False)
                    nc.tensor.matmul(po[:, FB:N], och, wos[k][:, FB:N],
                                     start=(k == 0), stop=False)
                po_parts[nch] = po

            wos = []

            def load_wo(k):
                wo = wvo.tile([P, D], BF16, tag="wvo")
                nc.gpsimd.dma_start(out=wo, in_=wout[ds(k * P, P), :])
                wos.append(wo)

            for pa in range(KT):
                fill = []
                if pa == 0:
                    for mc in range(NT):
                        fill.append(lambda mc=mc: emit_vproj_half(mc, 0))
                        fill.append(lambda mc=mc: emit_vproj_half(mc, 1))
                if pa + 1 < KT:
                    fill.extend(proj_fill(pa + 1))
                if pa == KT - 2:
                    fill.append(lambda: [load_wo(k) for k in range(KT)])
                if pa == KT - 1:
                    att(pa, fill,
                        pre_recip=lambda: (po_part(0), po_part(1)))
                else:
                    att(pa, fill)

            # ---------------- out projection ------------------------------
            for nch in range(NT):
                po = po_parts.pop(nch, None)
                if po is None:
                    po = spp.tile([P, N], FP32, tag="sp")
                    ks = range(KT)
                else:
                    ks = range(KT - 1, KT)
                for k in ks:
                    och = oT_sb[:, k, ds(nch * P, P)]
                    nc.tensor.matmul(po[:, 0:FB], och, wos[k][:, 0:FB],
                                     start=(k == 0), stop=False)
                    nc.tensor.matmul(po[:, FB:N], och, wos[k][:, FB:N],
                                     start=(k == 0), stop=False)
                nc.tensor.matmul(po[:, 0:FB], e0m, b128[:, 0:FB],
                                 start=False, stop=True)
                nc.tensor.matmul(po[:, FB:N], e0m, b128[:, FB:N],
                                 start=False, stop=True)
                for half in range(2):
                    osbt = osbp.tile([P, FB], FP32, tag="osb")
                    if half == 0:
                        nc.scalar.copy(out=osbt, in_=po[:, 0:FB])
                    else:
                        nc.vector.tensor_copy(out=osbt, in_=po[:, FB:N])
                    eng = (nc.sync, nc.gpsimd, nc.scalar)[(2 * nch + half) % 3]
                    eng.dma_start(out=out[ds(nch * P, P), ds(half * FB, FB)],
                                  in_=osbt)

    nc.compile()
    return nc


def _host_inputs(Wqkv, bqkv, Wout, bout, q_scale, k_scale):
    import ml_dtypes
    BF = ml_dtypes.bfloat16
    cosF, sinF = _build_tables()

    swapm = np.zeros((P, P), np.float32)
    for k in range(P):
        m = (k & ~63) + ((k & 63) ^ 32)
        swapm[k, m] = 1.0

    # Fold q/k_scale into the Q/K projection columns; the RMSNorm variance of
    # the *unscaled* q is then recovered with a 1/scale^2-weighted reduction.
    qs = q_scale.astype(np.float32)
    ks = k_scale.astype(np.float32)
    W = Wqkv.astype(np.float32).copy()
    b = bqkv.astype(np.float32).copy()
    qcol = np.tile(qs, H)
    kcol = np.tile(ks, H)
    W[:, 0:D] *= qcol[None, :]
    W[:, D:2 * D] *= kcol[None, :]
    b[0:D] *= qcol
    b[D:2 * D] *= kcol

    def wones(sv):
        o = np.zeros((P, 2), np.float32)
        inv2 = 1.0 / (sv * sv)
        o[0:HD, 0] = inv2
        o[HD:P, 1] = inv2
        return o

    bqkv_cols = np.ascontiguousarray(
        b[:2 * D].reshape(2 * KT, P).T).astype(np.float32)

    Wqk = W[:, :2 * D].astype(BF)
    # wqkt[t, ki, ko, f] = Wqk[ko*128+ki, t*128+f]
    wqkt = np.ascontiguousarray(
        Wqk.reshape(KT, P, 2 * KT, P).transpose(2, 1, 0, 3))

    return {
        "wqkt": wqkt,
        "wv": np.ascontiguousarray(W[:, 2 * D:]).astype(BF),
        "wout": Wout.astype(np.float32).astype(BF),
        "bqkv_cols": bqkv_cols,
        "bv_row": b[2 * D:].reshape(1, D).astype(BF),
        "bout_row": bout.reshape(1, D).astype(np.float32).astype(BF),
        "cosf": cosF.astype(BF), "sinf": sinF.astype(BF),
        "swapm": swapm.astype(BF),
        "ones2q": wones(qs).astype(BF), "ones2k": wones(ks).astype(BF),
        "ident": np.eye(P, dtype=np.float32).astype(BF),
    }


def _get_built():
    global _BUILT
    if _BUILT is None:
        _BUILT = _build_program()
    return _BUILT


def kernel(x, Wqkv, bqkv, Wout, bout, q_scale, k_scale, _trace=False):
    from concourse.bass_utils import run_bass_kernel_spmd

    import ml_dtypes
    x = np.asarray(x, dtype=np.float32).astype(ml_dtypes.bfloat16)
    shared = _host_inputs(np.asarray(Wqkv, np.float32), np.asarray(bqkv, np.float32),
                          np.asarray(Wout, np.float32), np.asarray(bout, np.float32),
                          np.asarray(q_scale, np.float32), np.asarray(k_scale, np.float32))
    in_maps = [dict(shared, x=np.ascontiguousarray(x[c])) for c in range(B)]
    nc = _get_built()
    res = run_bass_kernel_spmd(nc, in_maps, core_ids=list(range(B)), trace=_trace)
    out = np.stack([res.results[c]["out"] for c in range(B)], axis=0)
    kernel.last_exec_time_ns = res.exec_time_ns
    kernel.last_results = res
    return out
